# revision 1
# baseline (speedup 1.0000x reference)
"""Trainium2 Bass kernel for nn_BConvAttention2d (binary conv-attention).

Computation (see reference):
  bx  = sign(x)                                   [B,64,224,224]
  sa  = per-(channel,patch) depthwise 3x3 conv of bx with sign(patch_filters)
        (image split into 14x14 grid of 16x16 patches, each padded independently)
  bsa = sign(sa)
  out = conv2d(bsa, sign(output_filters), 3x3, pad 1)   [B,128,224,224]

All values are in {-1,0,+1}; every intermediate is a small integer, so bf16
inputs with fp32 PSUM accumulation reproduce the fp32 reference exactly.

Sharding: data-parallel over batch, one batch element per NeuronCore (8 cores).

Per-core layout:
  - x is streamed in 7 "sections": partitions = (strip parity, channel) where a
    strip is 16 consecutive image rows; free dim = 16 rows x 224 cols.
  - depthwise conv: one scalar_tensor_tensor (acc = bx_shifted * w + acc) per
    (section, patch column, tap); per-partition fp32 scalar = quantized filter
    tap; patch zero-padding realized by restricting the output AP to the valid
    sub-rectangle.
  - main conv: per 16-row strip, build [128, 18*226] bf16 tile: partitions
    0..63 = channels (rows shifted -1), 64..127 = channels (unshifted); 226-wide
    rows with zero guard columns. 6 matmuls per 2-row output tile accumulate
    kh in {0,1} (K=128) and kh=2 (K=64 at partition base 64) over kw in {0,1,2}.
"""

import numpy as np

import concourse.bass as bass
import concourse.tile as tile
from concourse import mybir
from concourse.bass_utils import run_bass_kernel_spmd

F32 = mybir.dt.float32
BF16 = mybir.dt.bfloat16
FP8 = mybir.dt.float8e4
SIGN = mybir.ActivationFunctionType.Sign
MULT = mybir.AluOpType.mult
ADD = mybir.AluOpType.add

C = 64          # input channels
CO = 128        # output channels
H = W = 224
PH = 16         # patch size
GW = 14         # patch grid
NSEC = 7        # sections of 2 strips (32 rows) each
NSTRIP = 14     # 16-row strips
SECF = PH * W       # free elems per section partition (16 rows x 224)
ROWP = W + 2        # padded row width in the strip tile (guard cols)

# patch-column split between DVE and GPSIMD for the depthwise stage
DVE_GW = 14

# strips that become computable after each section's depthwise finishes
STRIPS_AFTER = {0: [0], 1: [1, 2], 2: [3, 4], 3: [5, 6], 4: [7, 8],
                5: [9, 10], 6: [11, 12, 13]}


def _split_multiwait_ctrl(nc, max_waits=1):
    """walrus in this toolchain rejects instructions carrying more than one
    sync-wait; hoist extras onto preceding NOPs on the same engine."""
    n_new = 0
    for blk in nc.m.functions[0].blocks:
        new_insts = []
        changed = False
        for inst in blk.instructions:
            si = inst.sync_info
            if (si is not None and len(si.on_wait) > max_waits):
                waits = list(si.on_wait)
                for w in waits[: len(waits) - max_waits]:
                    nop = mybir.InstNoOp(name=f"I-splitwait-{n_new}", ins=[], outs=[])
                    n_new += 1
                    nop.engine = inst.engine
                    nop.sync_info = mybir.SyncInfo(on_wait=[w], on_update=[])
                    new_insts.append(nop)
                inst.sync_info = mybir.SyncInfo(
                    on_wait=waits[len(waits) - max_waits:],
                    on_update=list(si.on_update))
                changed = True
            new_insts.append(inst)
        if changed:
            blk.instructions = new_insts
    return n_new


def _emit_weights(nc, tc, pool, pf, of, psum_pool):
    """Quantize patch filters into per-section [128, 126] fp32 scalar tiles and
    output filters into transposed bf16 lhsT tiles w2a/w2b."""
    from concourse.masks import make_identity

    pfall = pool.tile([64, 196 * 9], F32, tag="pfall")
    nc.scalar.dma_start(pfall[:], pf.rearrange("c p kh kw -> c (p kh kw)"))
    wsec = []
    for s in range(NSEC):
        wt = pool.tile([128, GW * 9], F32, tag=f"wsec{s}")
        for hh in range(2):
            o = (28 * s + 14 * hh) * 9
            nc.scalar.dma_start(wt[64 * hh:64 * hh + 64, :], pfall[:, o:o + 126])
        nc.scalar.activation(wt[:], wt[:], SIGN)
        wsec.append(wt)

    # expanded per-column weight maps, one [128, W] bf16 tile per (section, tap)
    # with the within-patch column-validity mask baked in as zeros
    wexp = []
    for s in range(NSEC):
        per_tap = {}
        for t in range(9):
            u, v = divmod(t, 3)
            we = pool.tile([128, W], BF16, tag=f"wexp{s}_{t}")
            wv = wsec[s][:].rearrange("p (gw q) -> p gw q", gw=GW)[:, :, t].unsqueeze(2).broadcast_to([128, GW, PH])
            nc.scalar.activation(
                we[:].rearrange("p (gw q) -> p gw q", gw=GW), wv,
                mybir.ActivationFunctionType.Copy)
            if v == 0:
                nc.gpsimd.memset(
                    we[:].rearrange("p (gw q) -> p gw q", gw=GW)[:, :, PH - 1:PH],
                    0.0)
            elif v == 2:
                nc.gpsimd.memset(
                    we[:].rearrange("p (gw q) -> p gw q", gw=GW)[:, :, 0:1], 0.0)
            per_tap[(u, v)] = we
        wexp.append(per_tap)

    # output filters: natural load + sign, then PE-transpose into lhsT layout
    wof_nat = pool.tile([128, 576], F32, tag="wof_nat")
    nc.scalar.dma_start(wof_nat[:], of.rearrange("co ci kh kw -> co (ci kh kw)"))
    wofq = pool.tile([128, 576], BF16, tag="wofq")
    nc.scalar.activation(wofq[:], wof_nat[:], SIGN)
    ident = pool.tile([128, 128], BF16, tag="ident")
    make_identity(nc, ident[:])

    w2ab = pool.tile([128, 3 * CO], BF16, tag="w2ab")  # bf16 staging
    w2bb = pool.tile([128, 3 * CO], BF16, tag="w2bb")
    w2a = pool.tile([128, 3 * CO], FP8, tag="w2a")   # [(kh01, ci), (kw, co)]
    w2b = pool.tile([128, 3 * CO], FP8, tag="w2b")   # kh=2 at partitions 64:128
    wofq3 = wofq[:].rearrange("co (ci u) -> co ci u", u=9)
    if True:
        for kh in range(3):
            for kw in range(3):
                pt = psum_pool.tile([64, CO], BF16, tag="ps", bufs=6,
                                    name=f"ptr_{kh}_{kw}")
                nc.tensor.transpose(pt[:], wofq3[:, :, 3 * kh + kw], ident[:])
                if kh < 2:
                    dst = w2ab[64 * kh:64 * kh + 64, kw * CO:(kw + 1) * CO]
                else:
                    dst = w2bb[0:64, kw * CO:(kw + 1) * CO]
                nc.scalar.copy(dst, pt[:])
    nc.vector.tensor_copy(w2a[:], w2ab[:])
    nc.vector.tensor_copy(w2b[0:64, :], w2bb[0:64, :])
    nc.scalar.dma_start(w2b[64:128, :], w2b[0:64, :])
    return wexp, w2a, w2b, ident


# tap -> (dense flat out_lo/out_hi, in_lo) for acc += shift(tmp_t);
# boundary elements clipped off always have zero true contribution, and all
# row-wrap reads land on columns the wexp mask zeroed.
TAP_RANGE = {
    (0, 0): (225, SECF, 0),
    (0, 1): (224, SECF, 0),
    (0, 2): (224, SECF, 1),
    (1, 0): (1, SECF, 0),
    (1, 2): (0, SECF - 1, 1),
    (2, 0): (0, SECF - 224, 223),
    (2, 1): (0, SECF - 224, 224),
    (2, 2): (0, SECF - 225, 225),
}
PE_TAPS = [(1, 1), (1, 0), (1, 2), (0, 0)]  # summed on TensorE (identity mms)
DVE_TAPS = [(0, 1), (2, 1), (2, 0)]     # TT adds on VectorE
GP_TAPS = [(0, 2), (2, 2)]              # TT adds on GpSimd
NCHUNK = 8
CHUNK = SECF // NCHUNK


def _emit_mults(nc, s, xs, bxs, wexp_s, tmp_pool):
    """sign(x section) then the 9 masked products tmp_t = bx * wexp_t."""
    nc.scalar.activation(bxs[:], xs[:], SIGN)
    bx3 = bxs[:].rearrange("p (ph w) -> p ph w", ph=PH)
    tmps = {}
    for k, t in enumerate(PE_TAPS + DVE_TAPS + GP_TAPS):
        tm = tmp_pool.tile([128, SECF], BF16, tag="tmp", name=f"tmp_{s}_{t[0]}{t[1]}")
        wb = wexp_s[t][:].unsqueeze(1).broadcast_to([128, PH, W])
        eng = nc.gpsimd if k == len(PE_TAPS + DVE_TAPS + GP_TAPS) - 1 else nc.vector
        eng.tensor_tensor(
            tm[:].rearrange("p (ph w) -> p ph w", ph=PH), bx3, wb, MULT)
        tmps[t] = tm
    return tmps


def _emit_tapsum(nc, s, tmps, acc, ident, psum_pool):
    """Tap summation: PE taps into PSUM chunks + evict, then DVE/GP adds."""
    # PE: psum[chunk] = tmp_11 + shifted PE taps, evict to acc
    for c in range(NCHUNK):
        o0, o1 = c * CHUNK, (c + 1) * CHUNK
        dwp = psum_pool.tile([128, CHUNK], F32, tag="dwps", bufs=2,
                             name=f"dwp_{s}_{c}")
        nc.tensor.matmul(dwp[:], ident[:], tmps[(1, 1)][:, o0:o1],
                         start=True, stop=False)
        shift_taps = [t for t in PE_TAPS if t != (1, 1)]
        for k, t in enumerate(shift_taps):
            tlo, thi, tilo = TAP_RANGE[t]
            lo, hi = max(o0, tlo), min(o1, thi)
            delta = tilo - tlo
            nc.tensor.matmul(dwp[:, lo - o0:hi - o0], ident[:],
                             tmps[t][:, lo + delta:hi + delta],
                             start=False, stop=(k == len(shift_taps) - 1))
        nc.scalar.copy(acc[:, o0:o1], dwp[:])
    for t in DVE_TAPS:
        lo, hi, ilo = TAP_RANGE[t]
        nc.vector.tensor_tensor(acc[:, lo:hi], tmps[t][:, ilo:ilo + hi - lo],
                                acc[:, lo:hi], ADD)
    for t in GP_TAPS:
        lo, hi, ilo = TAP_RANGE[t]
        nc.gpsimd.tensor_tensor(acc[:, lo:hi], tmps[t][:, ilo:ilo + hi - lo],
                                acc[:, lo:hi], ADD)


def _emit_bsa2(nc, g, bsaq, bsa2_pool):
    """Build the strip's [128, 18*226] fp8 tile (dup+shift layout, guards)."""
    t2 = bsa2_pool.tile([128, 18 * ROWP], FP8, tag="bsa2", name=f"bsa2_{g}")
    t3 = t2[:].rearrange("p (r w) -> p r w", r=18)
    nc.gpsimd.memset(t3[:, :, 0:1], 0.0)
    nc.gpsimd.memset(t3[:, :, ROWP - 1:ROWP], 0.0)
    if g == 0:
        nc.gpsimd.memset(t3[0:64, 0:1, :], 0.0)
    if g == NSTRIP - 1:
        nc.gpsimd.memset(t3[64:128, 16:17, :], 0.0)

    sg, hb = divmod(g, 2)
    hb *= 64
    bq3 = bsaq[sg][:].rearrange("p (ph w) -> p ph w", ph=PH)
    # half0 (partitions 0:64): image rows 16g-1 .. 16g+14 in slots 0..15
    nc.sync.dma_start(t3[0:64, 1:16, 1:W + 1], bq3[hb:hb + 64, 0:15, :])
    if g > 0:
        sp, hp = divmod(g - 1, 2)
        bp3 = bsaq[sp][:].rearrange("p (ph w) -> p ph w", ph=PH)
        nc.sync.dma_start(t3[0:64, 0:1, 1:W + 1],
                          bp3[hp * 64:hp * 64 + 64, 15:16, :])
    # half1 (partitions 64:128): image rows 16g .. 16g+16 in slots 0..16
    nc.sync.dma_start(t3[64:128, 0:16, 1:W + 1], bq3[hb:hb + 64, 0:16, :])
    if g < NSTRIP - 1:
        sn, hn = divmod(g + 1, 2)
        bn3 = bsaq[sn][:].rearrange("p (ph w) -> p ph w", ph=PH)
        nc.sync.dma_start(t3[64:128, 16:17, 1:W + 1],
                          bn3[hn * 64:hn * 64 + 64, 0:1, :])
    return t2


def _dr_rhs(t2, base_part, npart, offset):
    """Overlapping [K, 2, 2, 224] fp8 rhs: kw-pair x 2 rows at a free offset."""
    import concourse.bass as bass
    a = t2[base_part:base_part + npart, :]
    return bass.AP(tensor=a.tensor, offset=a.offset + offset,
                   ap=[list(a.ap[0]), [1, 2], [ROWP, 2], [1, W]])


def _emit_strip(nc, tc, g, t2, w2a, w2b, y, out_pool, psum_pool):
    """Main conv for one 16-row strip: fp8, 4 matmuls per 2-row tile."""
    t3 = t2[:].rearrange("p (r w) -> p r w", r=18)
    w2a_dr = w2a[:, 0:2 * CO].rearrange("p (r m) -> p r m", r=2)
    w2b_dr = w2b[64:128, 0:2 * CO].rearrange("p (r m) -> p r m", r=2)
    for rh in range(2):
        rps = range(4 * rh, 4 * rh + 4)
        ps = {rp: psum_pool.tile([CO, 2 * W], F32, tag="ps", bufs=6,
                                 name=f"ps_{g}_{rp}") for rp in rps}
        ps3 = {rp: ps[rp][:].rearrange("p (r w) -> p r w", r=2) for rp in rps}
        for wi in range(4):
            for rp in rps:
                if wi == 0:     # kh01 x kw01, DoubleRow, K=128
                    nc.tensor.matmul(ps3[rp], w2a_dr,
                                     _dr_rhs(t2, 0, 128, 2 * rp * ROWP),
                                     start=True, stop=False,
                                     perf_mode=mybir.MatmulPerfMode.DoubleRow)
                elif wi == 1:   # kh01 x kw2, K=128
                    nc.tensor.matmul(ps3[rp], w2a[:, 2 * CO:3 * CO],
                                     t3[0:128, 2 * rp:2 * rp + 2, 2:2 + W],
                                     start=False, stop=False)
                elif wi == 2:   # kh2 x kw01, DoubleRow, K=64 at base 64
                    nc.tensor.matmul(ps3[rp], w2b_dr,
                                     _dr_rhs(t2, 64, 64, (2 * rp + 1) * ROWP),
                                     start=False, stop=False,
                                     perf_mode=mybir.MatmulPerfMode.DoubleRow)
                else:           # kh2 x kw2, K=64
                    nc.tensor.matmul(ps3[rp], w2b[64:128, 2 * CO:3 * CO],
                                     t3[64:128, 2 * rp + 1:2 * rp + 3, 2:2 + W],
                                     start=False, stop=True)
        for rp in rps:
            ob = out_pool.tile([CO, 2 * W], F32)
            nc.scalar.copy(ob[:], ps[rp][:])
            nc.scalar.dma_start(y[:, 16 * g + 2 * rp:16 * g + 2 * rp + 2, :],
                                ob[:].rearrange("p (r w) -> p r w", r=2))


def _build_program():
    nc = bass.Bass()
    x = nc.declare_dram_parameter("x", [C, H, W], F32, isOutput=False)
    pf = nc.declare_dram_parameter("pf", [C, 196, 3, 3], F32, isOutput=False)
    of = nc.declare_dram_parameter("of", [CO, C, 3, 3], F32, isOutput=False)
    y = nc.declare_dram_parameter("y", [CO, H, W], F32, isOutput=True)

    with tile.TileContext(nc) as tc:
        with tc.tile_pool(name="wpool", bufs=1) as wpool, \
             tc.tile_pool(name="xpool", bufs=2) as xpool, \
             tc.tile_pool(name="bxpool", bufs=2) as bxpool, \
             tc.tile_pool(name="bsaq", bufs=1) as bsaq_pool, \
             tc.tile_pool(name="bsa2", bufs=3) as bsa2_pool, \
             tc.tile_pool(name="accp", bufs=2) as acc_pool, \
             tc.tile_pool(name="tmp", bufs=11) as tmp_pool, \
             tc.tile_pool(name="outsb", bufs=6) as out_pool, \
             tc.tile_pool(name="psum", bufs=8, space="PSUM") as psum_pool:
            wexp, w2a, w2b, ident = _emit_weights(nc, tc, wpool, pf, of,
                                                  psum_pool)
            bsaq = []
            tmps_by_s = {}

            def stage_front(s):
                xs = xpool.tile([128, SECF], BF16, tag="xs", name=f"xs_{s}")
                for hh in range(2):
                    src = x[:, 32 * s + 16 * hh:32 * s + 16 * hh + 16, :].rearrange(
                        "c ph w -> c (ph w)")
                    nc.gpsimd.dma_start(xs[64 * hh:64 * hh + 64, :], src)
                bxs = bxpool.tile([128, SECF], BF16, tag="bxs", name=f"bxs_{s}")
                tmps_by_s[s] = _emit_mults(nc, s, xs, bxs, wexp[s], tmp_pool)

            def stage_tapsum(s):
                acc = acc_pool.tile([128, SECF], BF16, tag="acc", bufs=2,
                                    name=f"acc_{s}")
                bq = bsaq_pool.tile([128, SECF], FP8, tag="bsaq", bufs=3,
                                    name=f"bsaq_{s}")
                bsaq.append(bq)
                _emit_tapsum(nc, s, tmps_by_s.pop(s), acc, ident, psum_pool)
                nc.scalar.activation(bq[:], acc[:], SIGN)  # bsa -> fp8

            stage_front(0)
            stage_front(1)
            for s in range(NSEC):
                stage_tapsum(s)
                t2s = [(g, _emit_bsa2(nc, g, bsaq, bsa2_pool))
                       for g in STRIPS_AFTER[s]]
                if s + 2 < NSEC:
                    stage_front(s + 2)
                for g, t2 in t2s:
                    _emit_strip(nc, tc, g, t2, w2a, w2b, y, out_pool, psum_pool)
    _split_multiwait_ctrl(nc)
    return nc


_PROGRAM_CACHE = {}


def _get_program():
    if "nc" not in _PROGRAM_CACHE:
        _PROGRAM_CACHE["nc"] = _build_program()
    return _PROGRAM_CACHE["nc"]


def run(x, a, patch_filters, output_filters, trace=False):
    """Run on 8 NeuronCores (batch-parallel); returns (out, results_obj)."""
    nc = _get_program()
    pf = np.ascontiguousarray(
        np.asarray(patch_filters, np.float32).reshape(C, 196, 3, 3))
    of = np.ascontiguousarray(np.asarray(output_filters, np.float32))
    xs = np.asarray(x, np.float32)
    in_maps = [{"x": np.ascontiguousarray(xs[b]), "pf": pf, "of": of}
               for b in range(8)]
    res = run_bass_kernel_spmd(nc, in_maps, list(range(8)), trace=trace)
    out = np.stack([res.results[b]["y"] for b in range(8)])
    return out, res


def kernel(x, a, patch_filters, output_filters):
    out, _ = run(x, a, patch_filters, output_filters)
    return out



# revision 31
# speedup vs baseline: 1.8042x; 1.8042x over previous
"""Trainium2 Bass kernel for nn_BConvAttention2d (binary conv-attention).

Computation (see reference):
  bx  = sign(x)                                   [B,64,224,224]
  sa  = per-(channel,patch) depthwise 3x3 conv of bx with sign(patch_filters)
        (image split into 14x14 grid of 16x16 patches, each padded independently)
  bsa = sign(sa)
  out = conv2d(bsa, sign(output_filters), 3x3, pad 1)   [B,128,224,224]

Everything is {-1,0,+1}; intermediates are small integers, so fp8/bf16 inputs
with fp32 PSUM accumulation reproduce the fp32 reference exactly (fp16 output
holds ints <= 2048 exactly).

Sharding: data-parallel over batch, one batch element per NeuronCore (8 cores).

Per-core design:
  - x arrives host-pre-tiled: 7 "sections" of [128, 3584] bf16, partitions =
    (strip parity, channel), free = 16 rows x 224 cols (a strip = one patch row).
  - bxs = sign(x) as fp8 (ACT); bxL/bxR = masked copies (int32 tensor_copy +
    tiny memsets) zeroing the patch-wrap columns for kw=0 / kw=2 taps.
  - depthwise products: 9 per section via ONE fused DVE scalar_tensor_tensor
    each: tmp_t = (w_raw_fp8 & 0x80808080) ^ bx  (sign-flip multiply, int32
    views over fp8 bytes). Products land in a zero-guarded 9-slot tile.
  - tap summation on PE: per 448-elem chunk, 4 DoubleRow identity matmuls
    (pairing taps; guards+masks make full-range APs valid) + 1 single, then a
    fused ACT Sign eviction PSUM -> bsaq fp8. No bf16 accumulator.
  - main conv per 16-row strip from a [128, 18*226] fp8 dup+shift tile:
    3 matmuls per 2-row output tile: DR(kh01 x kw01, K=128),
    DR(kh01-kw2, kh2-kw0; K=128, upper pair-weights zero), DR(kh2-kw1,
    kh2-kw2; K=64 at partitions 64:128). Evictions to fp16 (DVE+ACT split),
    one batched DMA per strip.
  - weights are host-packed raw values (pure gather/cast; sign quantization
    happens on device).
"""

import numpy as np
import ml_dtypes

import concourse.bass as bass
import concourse.tile as tile
from concourse import mybir
from concourse.bass_utils import run_bass_kernel_spmd

F32 = mybir.dt.float32
F16 = mybir.dt.float16
BF16 = mybir.dt.bfloat16
FP8 = mybir.dt.float8e4
I32 = mybir.dt.int32
SIGN = mybir.ActivationFunctionType.Sign
XOR = mybir.AluOpType.bitwise_xor
AND = mybir.AluOpType.bitwise_and

C = 64          # input channels
CO = 128        # output channels
H = W = 224
PH = 16         # patch size
GW = 14         # patch grid width
NSEC = 7        # sections of 2 strips (32 rows) each
NSTRIP = 14     # 16-row strips
SECF = PH * W   # 3584 free elems per section partition
ROWP = W + 2    # padded row width in the strip tile
CH = 448        # tapsum chunk (2 rows)
NCHUNK = SECF // CH

GUARD = 240
SLOT = SECF + GUARD          # 3824, guarded product slot stride
SLOT_BASE = [GUARD + k * SLOT for k in range(9)]
TMPSZ = SLOT_BASE[8] + SLOT
MASK_I32 = -2139062144       # 0x80808080: fp8 sign bits

# product slots: [(u, v), slot]; DR pairs: (0,1), (2,3), (4,5), (6,7); single 8
TAPS = [((0, 0), 0), ((2, 0), 1), ((0, 1), 2), ((2, 1), 3),
        ((0, 2), 4), ((2, 2), 5), ((1, 0), 6), ((1, 2), 7), ((1, 1), 8)]
DELTA = {(u, v): 224 * (u - 1) + (v - 1) for u in range(3) for v in range(3)}

# strips that become computable after each section's depthwise finishes
STRIPS_AFTER = {0: [0], 1: [1, 2], 2: [3, 4], 3: [5, 6], 4: [7, 8],
                5: [9, 10], 6: [11, 12, 13]}


def _split_multiwait_ctrl(nc, max_waits=1):
    """walrus in this toolchain rejects instructions carrying more than one
    sync-wait; hoist extras onto preceding NOPs on the same engine."""
    n_new = 0
    for blk in nc.m.functions[0].blocks:
        new_insts = []
        changed = False
        for inst in blk.instructions:
            si = inst.sync_info
            if (si is not None and len(si.on_wait) > max_waits):
                waits = list(si.on_wait)
                for w in waits[: len(waits) - max_waits]:
                    nop = mybir.InstNoOp(name=f"I-splitwait-{n_new}", ins=[], outs=[])
                    n_new += 1
                    nop.engine = inst.engine
                    nop.sync_info = mybir.SyncInfo(on_wait=[w], on_update=[])
                    new_insts.append(nop)
                inst.sync_info = mybir.SyncInfo(
                    on_wait=waits[len(waits) - max_waits:],
                    on_update=list(si.on_update))
                changed = True
            new_insts.append(inst)
        if changed:
            blk.instructions = new_insts
    return n_new


def _slot_ap(tmp, slot, extra, shape_ap):
    """AP into the guarded product tile at slot base + extra offset."""
    return bass.AP(tensor=tmp[:].tensor,
                   offset=tmp[:].offset + SLOT_BASE[slot] + extra,
                   ap=[list(tmp[:].ap[0])] + shape_ap)


def _build_program():
    nc = bass.Bass()
    xt = nc.declare_dram_parameter("xt", [NSEC, 128, SECF], BF16, isOutput=False)
    wdw = nc.declare_dram_parameter("wdw", [128, NSEC * 9 * W], FP8, isOutput=False)
    w2 = nc.declare_dram_parameter("w2", [128, 768], BF16, isOutput=False)
    id2 = nc.declare_dram_parameter("id2", [128, 256], FP8, isOutput=False)
    id1 = nc.declare_dram_parameter("id1", [128, 128], FP8, isOutput=False)
    y = nc.declare_dram_parameter("y", [NSTRIP, CO, 8 * CH], F16, isOutput=True)

    with tile.TileContext(nc) as tc:
        with tc.tile_pool(name="wpool", bufs=1) as wpool, \
             tc.tile_pool(name="xpool", bufs=2) as xpool, \
             tc.tile_pool(name="bxpool", bufs=2) as bxpool, \
             tc.tile_pool(name="tmpp", bufs=2) as tmp_pool, \
             tc.tile_pool(name="bsaq", bufs=3) as bsaq_pool, \
             tc.tile_pool(name="bsa2", bufs=3) as bsa2_pool, \
             tc.tile_pool(name="outsb", bufs=2) as out_pool, \
             tc.tile_pool(name="pstap", bufs=2, space="PSUM") as pstap_pool, \
             tc.tile_pool(name="psconv", bufs=6, space="PSUM") as psconv_pool:

            # ---- weights / constants ----
            wall = wpool.tile([128, NSEC * 9 * W], FP8, tag="wall")
            nc.scalar.dma_start(wall[:], wdw[:, :])
            w2r = wpool.tile([128, 768], BF16, tag="w2r")
            nc.scalar.dma_start(w2r[:], w2[:, :])
            w2q = wpool.tile([128, 768], FP8, tag="w2q")
            nc.scalar.activation(w2q[:], w2r[:], SIGN)
            id2t = wpool.tile([128, 256], FP8, tag="id2")
            nc.scalar.dma_start(id2t[:], id2[:, :])
            id1t = wpool.tile([128, 128], FP8, tag="id1")
            nc.scalar.dma_start(id1t[:], id1[:, :])
            maskt = wpool.tile([128, 1], I32, tag="mask")
            nc.gpsimd.memset(maskt[:], MASK_I32)

            id2dr = id2t[:].rearrange("p (r m) -> p r m", r=2)
            # conv weights: w2a_dr = kh01 x (kw0,kw1) pairs; w2b2_dr =
            # (kh01-kw2, kh2-kw0) pairs (upper pair-weights zero); kh2-kw1/2
            # as K=64 singles at partitions 64:128.
            w2a_dr = w2q[:, 0:256].rearrange("p (r m) -> p r m", r=2)
            w2b2_dr = w2q[:, 256:512].rearrange("p (r m) -> p r m", r=2)
            w2c1 = w2q[64:128, 512:640]
            w2c2 = w2q[64:128, 640:768]

            bsaq = []
            tmps_by_s = {}

            def stage_front(s):
                xs = xpool.tile([128, SECF], BF16, tag="xs", name=f"xs_{s}")
                nc.sync.dma_start(xs[:], xt[s, :, :])
                bxs = bxpool.tile([128, SECF], FP8, tag="bxs", name=f"bxs_{s}")
                nc.scalar.activation(bxs[:], xs[:], SIGN)
                # bxl/bxr: masked copies zeroing the patch-wrap columns for
                # kw=0 / kw=2 taps (int32 views over fp8 bytes).
                bxl = bxpool.tile([128, SECF], FP8, tag="sl", name=f"sl_{s}")
                bxr = bxpool.tile([128, SECF], FP8, tag="sr", name=f"sr_{s}")
                nc.vector.tensor_copy(bxl[:].bitcast(I32), bxs[:].bitcast(I32))
                nc.vector.tensor_copy(bxr[:].bitcast(I32), bxs[:].bitcast(I32))
                bxl3 = bxl[:].rearrange("p (rg q) -> p rg q", q=PH)
                bxr3 = bxr[:].rearrange("p (rg q) -> p rg q", q=PH)
                nc.gpsimd.memset(bxl3[:, :, PH - 1:PH], 0.0)
                nc.gpsimd.memset(bxr3[:, :, 0:1], 0.0)
                sl, sr = bxl, bxr

                tmp = tmp_pool.tile([128, TMPSZ], FP8, tag="tmp",
                                    name=f"tmp_{s}")
                nc.gpsimd.memset(tmp[:, 0:GUARD], 0.0)
                for k in range(9):
                    gap_end = SLOT_BASE[k + 1] if k < 8 else TMPSZ
                    nc.gpsimd.memset(
                        tmp[:, SLOT_BASE[k] + SECF:gap_end], 0.0)

                srcs = {0: sl, 1: bxs, 2: sr}
                for (u, v), slot in TAPS:
                    woff = (s * 9 + slot) * W
                    wv = wall[:, woff:woff + W].bitcast(I32).unsqueeze(1) \
                        .broadcast_to([128, PH, W // 4])
                    sv = srcs[v][:].rearrange("p (r w) -> p r w", r=PH) \
                        .bitcast(I32)
                    dst = _slot_ap(tmp, slot, 0, [[W, PH], [1, W]]).bitcast(I32)
                    nc.vector.scalar_tensor_tensor(dst, wv, maskt[:, 0:1], sv,
                                                   AND, XOR)
                tmps_by_s[s] = tmp

            def stage_tapsum(s):
                tmp = tmps_by_s.pop(s)
                bq = bsaq_pool.tile([128, SECF], FP8, tag="bsaq", bufs=3,
                                    name=f"bsaq_{s}")
                bsaq.append(bq)
                for c in range(NCHUNK):
                    o0 = c * CH
                    ps = pstap_pool.tile([128, CH], F32, tag="dwps", bufs=2,
                                         name=f"dwp_{s}_{c}")
                    first = True
                    # only the even-base DR pair B ((0,1),(2,1)); odd-base
                    # pairs intermittently misread, so those taps are singles
                    for a in (2,):
                        (ua, va), _ = TAPS[a]
                        (ub, vb), _ = TAPS[a + 1]
                        da, db = DELTA[(ua, va)], DELTA[(ub, vb)]
                        step = SLOT_BASE[a + 1] - SLOT_BASE[a] + db - da
                        rhs = _slot_ap(tmp, a, da + o0, [[step, 2], [1, CH]])
                        nc.tensor.matmul(ps[:], id2dr, rhs, start=first,
                                         stop=False,
                                         perf_mode=mybir.MatmulPerfMode.DoubleRow)
                        first = False
                    for slot in (0, 1, 4, 5, 6, 7, 8):
                        (u, v), _ = TAPS[slot]
                        rhs1 = _slot_ap(tmp, slot, DELTA[(u, v)] + o0,
                                        [[1, CH]])
                        nc.tensor.matmul(ps[:], id1t[:], rhs1, start=first,
                                         stop=(slot == 8))
                        first = False
                    nc.scalar.activation(bq[:, o0:o0 + CH], ps[:], SIGN)

            def emit_bsa2(g):
                """[128, 18*226] fp8 strip tile: half0 rows -1..16 (slots
                0..17), half1 rows 0..16 (slots 0..16), zero guard cols."""
                t2 = bsa2_pool.tile([128, 18 * ROWP], FP8, tag="bsa2",
                                    name=f"bsa2_{g}")
                t3 = t2[:].rearrange("p (r w) -> p r w", r=18)
                nc.gpsimd.memset(t3[:, :, 0:1], 0.0)
                nc.gpsimd.memset(t3[:, :, ROWP - 1:ROWP], 0.0)
                sg, hb = divmod(g, 2)
                hb *= 64
                bq3 = bsaq[sg][:].rearrange("p (ph w) -> p ph w", ph=PH)
                nc.sync.dma_start(t3[0:64, 1:17, 1:W + 1], bq3[hb:hb + 64, :, :])
                nc.gpsimd.dma_start(t3[64:128, 0:16, 1:W + 1],
                                    bq3[hb:hb + 64, :, :])
                if g > 0:
                    sp, hp = divmod(g - 1, 2)
                    bp3 = bsaq[sp][:].rearrange("p (ph w) -> p ph w", ph=PH)
                    nc.sync.dma_start(t3[0:64, 0:1, 1:W + 1],
                                      bp3[hp * 64:hp * 64 + 64, 15:16, :])
                else:
                    nc.gpsimd.memset(t3[0:64, 0:1, :], 0.0)
                if g < NSTRIP - 1:
                    sn, hn = divmod(g + 1, 2)
                    bn3 = bsaq[sn][:].rearrange("p (ph w) -> p ph w", ph=PH)
                    nc.sync.dma_start(t3[0:64, 17:18, 1:W + 1],
                                      bn3[hn * 64:hn * 64 + 64, 0:1, :])
                    nc.gpsimd.dma_start(t3[64:128, 16:17, 1:W + 1],
                                        bn3[hn * 64:hn * 64 + 64, 0:1, :])
                else:
                    nc.gpsimd.memset(t3[0:64, 17:18, :], 0.0)
                    nc.gpsimd.memset(t3[64:128, 16:17, :], 0.0)
                # half1 slot 17 is read (x0 weight) by the kh2-kw0 pair MM;
                # zero it so stray NaN bytes can't poison PSUM via NaN*0.
                nc.gpsimd.memset(t3[64:128, 17:18, :], 0.0)
                return t2

            def emit_strip(g, t2):
                """Main conv for one strip: 4 fp8 matmuls per 2-row tile."""
                t3 = t2[:].rearrange("p (r w) -> p r w", r=18)
                ob = out_pool.tile([CO, 8 * CH], F16, tag="out", name=f"ob_{g}")
                for rh in range(2):
                    rps = range(4 * rh, 4 * rh + 4)
                    ps = {rp: psconv_pool.tile([CO, 2 * W], F32, tag="cps",
                                               bufs=6, name=f"ps_{g}_{rp}")
                          for rp in rps}
                    ps3 = {rp: ps[rp][:].rearrange("p (r w) -> p r w", r=2)
                           for rp in rps}

                    def rhs4(base_p, npart, off, pair_step):
                        a = t2[base_p:base_p + npart, :]
                        return bass.AP(tensor=a.tensor, offset=a.offset + off,
                                       ap=[list(a.ap[0]), [pair_step, 2],
                                           [ROWP, 2], [1, W]])

                    for rp in rps:   # kh01 x kw01, K=128 DR
                        nc.tensor.matmul(ps3[rp], w2a_dr,
                                         rhs4(0, 128, 2 * rp * ROWP, 1),
                                         start=True, stop=False,
                                         perf_mode=mybir.MatmulPerfMode.DoubleRow)
                    for rp in rps:   # (kh01-kw2, kh2-kw0), K=128 DR
                        nc.tensor.matmul(ps3[rp], w2b2_dr,
                                         rhs4(0, 128, 2 * rp * ROWP + 2,
                                              2 * ROWP - 2),
                                         start=False, stop=False,
                                         perf_mode=mybir.MatmulPerfMode.DoubleRow)
                    for rp in rps:   # kh2-kw1, K=64 single
                        nc.tensor.matmul(ps3[rp], w2c1,
                                         t3[64:128, 2 * rp + 1:2 * rp + 3,
                                            1:1 + W],
                                         start=False, stop=False)
                    for rp in rps:   # kh2-kw2, K=64 single
                        nc.tensor.matmul(ps3[rp], w2c2,
                                         t3[64:128, 2 * rp + 1:2 * rp + 3,
                                            2:2 + W],
                                         start=False, stop=True)
                    for rp in rps:
                        dst = ob[:, rp * CH:(rp + 1) * CH]
                        if rp in (0, 4):
                            nc.scalar.copy(dst, ps[rp][:])
                        else:
                            nc.vector.tensor_copy(dst, ps[rp][:])
                nc.gpsimd.dma_start(y[g, :, :], ob[:])

            stage_front(0)
            stage_front(1)
            for s in range(NSEC):
                stage_tapsum(s)
                t2s = [(g, emit_bsa2(g)) for g in STRIPS_AFTER[s]]
                if s + 2 < NSEC:
                    stage_front(s + 2)
                for g, t2 in t2s:
                    emit_strip(g, t2)
    _split_multiwait_ctrl(nc)
    return nc


def _host_prep(x, patch_filters, output_filters):
    """Pure layout/gather/cast prep of raw values (no arithmetic)."""
    B = x.shape[0]
    # x: [B, 64, 224, 224] -> per-core [7, 128, 3584] bf16
    xr = np.asarray(x, np.float32).reshape(B, C, NSEC, 2, PH, W)
    xt = np.ascontiguousarray(xr.transpose(0, 2, 3, 1, 4, 5)) \
        .reshape(B, NSEC, 128, SECF).astype(ml_dtypes.bfloat16)

    # depthwise weights: [64, 196, 3, 3] raw -> fp8 sign-preserving cast,
    # expanded per column: wdw[p=(h,c), (s,t_slot,col)]
    pf = np.asarray(patch_filters, np.float32).reshape(C, GW, GW, 3, 3)
    wdw = np.zeros((128, NSEC, 9, GW, PH), np.float32)
    for (u, v), slot in TAPS:
        # [c, gy, gx] -> [s, h, c, gx]
        wv = pf[:, :, :, u, v].reshape(C, NSEC, 2, GW).transpose(1, 2, 0, 3)
        wdw[:, :, slot, :, 0] = wv.reshape(NSEC, 128, GW).transpose(1, 0, 2)
    wdw[:] = wdw[:, :, :, :, 0:1]          # expand over the 16 patch cols
    wdw = wdw.reshape(128, NSEC * 9 * W).astype(ml_dtypes.float8_e4m3)

    # main-conv weights (m2 layout)
    of = np.asarray(output_filters, np.float32)
    w2 = np.zeros((128, 6, 128), np.float32)
    for kh in range(2):
        for c in range(C):
            p = 64 * kh + c
            w2[p, 0, :] = of[:, c, kh, 0]      # w2a_dr pair elem 0 (kw0)
            w2[p, 1, :] = of[:, c, kh, 1]      # w2a_dr pair elem 1 (kw1)
            w2[p, 2, :] = of[:, c, kh, 2]      # w2b2_dr elem 0 (kw2)
            w2[p, 3, :] = of[:, c, 2, 0] if kh == 0 else 0.0  # elem 1 (kh2-kw0)
    for c in range(C):
        w2[64 + c, 4, :] = of[:, c, 2, 1]      # kh2-kw1 single
        w2[64 + c, 5, :] = of[:, c, 2, 2]      # kh2-kw2 single
    w2 = w2.reshape(128, 768).astype(ml_dtypes.bfloat16)

    id1 = np.eye(128, dtype=np.float32).astype(ml_dtypes.float8_e4m3)
    id2 = np.zeros((128, 2, 128), np.float32)
    for k in range(128):
        id2[k, :, k] = 1.0
    id2 = id2.reshape(128, 256).astype(ml_dtypes.float8_e4m3)

    in_maps = [{"xt": np.ascontiguousarray(xt[b]), "wdw": wdw, "w2": w2,
                "id2": id2, "id1": id1} for b in range(B)]
    return in_maps


_PROGRAM_CACHE = {}


def _get_program():
    if "nc" not in _PROGRAM_CACHE:
        _PROGRAM_CACHE["nc"] = _build_program()
    return _PROGRAM_CACHE["nc"]


def run(x, a, patch_filters, output_filters, trace=False):
    """Run on 8 NeuronCores (batch-parallel); returns (out, results_obj)."""
    nc = _get_program()
    in_maps = _host_prep(x, patch_filters, output_filters)
    res = run_bass_kernel_spmd(nc, in_maps, list(range(8)), trace=trace)
    outs = []
    for b in range(8):
        yb = np.asarray(res.results[b]["y"], np.float32)  # [14, 128, 8*448]
        yb = yb.reshape(NSTRIP, CO, 8, 2, W).transpose(1, 0, 2, 3, 4)
        outs.append(yb.reshape(CO, H, W))
    return np.stack(outs), res


def kernel(x, a, patch_filters, output_filters):
    out, _ = run(x, a, patch_filters, output_filters)
    return out


# revision 35
# speedup vs baseline: 1.8242x; 1.0111x over previous
"""Trainium2 Bass kernel for nn_BConvAttention2d (binary conv-attention).

Computation (see reference):
  bx  = sign(x)                                   [B,64,224,224]
  sa  = per-(channel,patch) depthwise 3x3 conv of bx with sign(patch_filters)
        (image split into 14x14 grid of 16x16 patches, each padded independently)
  bsa = sign(sa)
  out = conv2d(bsa, sign(output_filters), 3x3, pad 1)   [B,128,224,224]

Everything is {-1,0,+1}; intermediates are small integers, so fp8/bf16 inputs
with fp32 PSUM accumulation reproduce the fp32 reference exactly (fp16 output
holds ints <= 2048 exactly).

Sharding: data-parallel over batch, one batch element per NeuronCore (8 cores).

Per-core design:
  - x arrives host-pre-tiled: 7 "sections" of [128, 3584] bf16, partitions =
    (strip parity, channel), free = 16 rows x 224 cols (a strip = one patch row).
  - bxs = sign(x) as fp8 (ACT); bxL/bxR = masked copies (int32 tensor_copy +
    tiny memsets) zeroing the patch-wrap columns for kw=0 / kw=2 taps.
  - depthwise products: 9 per section via ONE fused DVE scalar_tensor_tensor
    each: tmp_t = (w_raw_fp8 & 0x80808080) ^ bx  (sign-flip multiply, int32
    views over fp8 bytes). Products land in a zero-guarded 9-slot tile.
  - tap summation on PE: per 448-elem chunk, 4 DoubleRow identity matmuls
    (pairing taps; guards+masks make full-range APs valid) + 1 single, then a
    fused ACT Sign eviction PSUM -> bsaq fp8. No bf16 accumulator.
  - main conv per 16-row strip from a [128, 18*226] fp8 dup+shift tile:
    3 matmuls per 2-row output tile: DR(kh01 x kw01, K=128),
    DR(kh01-kw2, kh2-kw0; K=128, upper pair-weights zero), DR(kh2-kw1,
    kh2-kw2; K=64 at partitions 64:128). Evictions to fp16 (DVE+ACT split),
    one batched DMA per strip.
  - weights are host-packed raw values (pure gather/cast; sign quantization
    happens on device).
"""

import numpy as np
import ml_dtypes

import concourse.bass as bass
import concourse.tile as tile
from concourse import mybir
from concourse.bass_utils import run_bass_kernel_spmd

F32 = mybir.dt.float32
F16 = mybir.dt.float16
BF16 = mybir.dt.bfloat16
FP8 = mybir.dt.float8e4
I32 = mybir.dt.int32
SIGN = mybir.ActivationFunctionType.Sign
XOR = mybir.AluOpType.bitwise_xor
AND = mybir.AluOpType.bitwise_and

C = 64          # input channels
CO = 128        # output channels
H = W = 224
PH = 16         # patch size
GW = 14         # patch grid width
NSEC = 7        # sections of 2 strips (32 rows) each
NSTRIP = 14     # 16-row strips
SECF = PH * W   # 3584 free elems per section partition
ROWP = W + 2    # padded row width in the strip tile
CH = 448        # tapsum chunk (2 rows)
NCHUNK = SECF // CH

GUARD = 240
SLOT = SECF + GUARD          # 3824, guarded product slot stride
SLOT_BASE = [GUARD + k * SLOT for k in range(9)]
TMPSZ = SLOT_BASE[8] + SLOT
MASK_I32 = -2139062144       # 0x80808080: fp8 sign bits

# product slots: [(u, v), slot]; DR pairs: (0,1), (2,3), (4,5), (6,7); single 8
TAPS = [((0, 0), 0), ((2, 0), 1), ((0, 1), 2), ((2, 1), 3),
        ((0, 2), 4), ((2, 2), 5), ((1, 0), 6), ((1, 2), 7), ((1, 1), 8)]
DELTA = {(u, v): 224 * (u - 1) + (v - 1) for u in range(3) for v in range(3)}

# strips that become computable after each section's depthwise finishes
STRIPS_AFTER = {0: [0], 1: [1, 2], 2: [3, 4], 3: [5, 6], 4: [7, 8],
                5: [9, 10], 6: [11, 12, 13]}


def _split_multiwait_ctrl(nc, max_waits=1):
    """walrus in this toolchain rejects instructions carrying more than one
    sync-wait; hoist extras onto preceding NOPs on the same engine."""
    n_new = 0
    for blk in nc.m.functions[0].blocks:
        new_insts = []
        changed = False
        for inst in blk.instructions:
            si = inst.sync_info
            if (si is not None and len(si.on_wait) > max_waits):
                waits = list(si.on_wait)
                for w in waits[: len(waits) - max_waits]:
                    nop = mybir.InstNoOp(name=f"I-splitwait-{n_new}", ins=[], outs=[])
                    n_new += 1
                    nop.engine = inst.engine
                    nop.sync_info = mybir.SyncInfo(on_wait=[w], on_update=[])
                    new_insts.append(nop)
                inst.sync_info = mybir.SyncInfo(
                    on_wait=waits[len(waits) - max_waits:],
                    on_update=list(si.on_update))
                changed = True
            new_insts.append(inst)
        if changed:
            blk.instructions = new_insts
    return n_new


def _slot_ap(tmp, slot, extra, shape_ap):
    """AP into the guarded product tile at slot base + extra offset."""
    return bass.AP(tensor=tmp[:].tensor,
                   offset=tmp[:].offset + SLOT_BASE[slot] + extra,
                   ap=[list(tmp[:].ap[0])] + shape_ap)


def _build_program():
    nc = bass.Bass()
    xt = nc.declare_dram_parameter("xt", [NSEC, 128, SECF], BF16, isOutput=False)
    wdw = nc.declare_dram_parameter("wdw", [128, NSEC * 9 * W], FP8, isOutput=False)
    w2 = nc.declare_dram_parameter("w2", [128, 768], BF16, isOutput=False)
    id2 = nc.declare_dram_parameter("id2", [128, 256], FP8, isOutput=False)
    id1 = nc.declare_dram_parameter("id1", [128, 128], FP8, isOutput=False)
    y = nc.declare_dram_parameter("y", [NSTRIP, CO, 8 * CH], F16, isOutput=True)

    with tile.TileContext(nc) as tc:
        with tc.tile_pool(name="wpool", bufs=1) as wpool, \
             tc.tile_pool(name="xpool", bufs=2) as xpool, \
             tc.tile_pool(name="bxpool", bufs=2) as bxpool, \
             tc.tile_pool(name="tmpp", bufs=2) as tmp_pool, \
             tc.tile_pool(name="bsaq", bufs=3) as bsaq_pool, \
             tc.tile_pool(name="bsa2", bufs=3) as bsa2_pool, \
             tc.tile_pool(name="outsb", bufs=2) as out_pool, \
             tc.tile_pool(name="pstap", bufs=2, space="PSUM") as pstap_pool, \
             tc.tile_pool(name="psconv", bufs=6, space="PSUM") as psconv_pool:

            # ---- weights / constants ----
            wall = wpool.tile([128, NSEC * 9 * W], FP8, tag="wall")
            nc.scalar.dma_start(wall[:], wdw[:, :])
            w2r = wpool.tile([128, 768], BF16, tag="w2r")
            nc.scalar.dma_start(w2r[:], w2[:, :])
            w2q = wpool.tile([128, 768], FP8, tag="w2q")
            nc.scalar.activation(w2q[:], w2r[:], SIGN)
            id2t = wpool.tile([128, 256], FP8, tag="id2")
            nc.scalar.dma_start(id2t[:], id2[:, :])
            id1t = wpool.tile([128, 128], FP8, tag="id1")
            nc.scalar.dma_start(id1t[:], id1[:, :])
            maskt = wpool.tile([128, 1], I32, tag="mask")
            nc.gpsimd.memset(maskt[:], MASK_I32)

            id2dr = id2t[:].rearrange("p (r m) -> p r m", r=2)
            # conv weights: w2a_dr = kh01 x (kw0,kw1) pairs; w2b2_dr =
            # (kh01-kw2, kh2-kw0) pairs (upper pair-weights zero); kh2-kw1/2
            # as K=64 singles at partitions 64:128.
            w2a_dr = w2q[:, 0:256].rearrange("p (r m) -> p r m", r=2)
            w2b2_dr = w2q[:, 256:512].rearrange("p (r m) -> p r m", r=2)
            w2c1 = w2q[64:128, 512:640]
            w2c2 = w2q[64:128, 640:768]

            bsaq = []
            tmps_by_s = {}

            def stage_front(s):
                xs = xpool.tile([128, SECF], BF16, tag="xs", name=f"xs_{s}")
                nc.sync.dma_start(xs[:], xt[s, :, :])
                bxs = bxpool.tile([128, SECF], FP8, tag="bxs", name=f"bxs_{s}")
                nc.scalar.activation(bxs[:], xs[:], SIGN)
                # bxl/bxr: masked copies zeroing the patch-wrap columns for
                # kw=0 / kw=2 taps (int32 views over fp8 bytes).
                bxl = bxpool.tile([128, SECF], FP8, tag="sl", name=f"sl_{s}")
                bxr = bxpool.tile([128, SECF], FP8, tag="sr", name=f"sr_{s}")
                nc.vector.tensor_copy(bxl[:].bitcast(I32), bxs[:].bitcast(I32))
                nc.vector.tensor_copy(bxr[:].bitcast(I32), bxs[:].bitcast(I32))
                bxl3 = bxl[:].rearrange("p (rg q) -> p rg q", q=PH)
                bxr3 = bxr[:].rearrange("p (rg q) -> p rg q", q=PH)
                nc.gpsimd.memset(bxl3[:, :, PH - 1:PH], 0.0)
                nc.gpsimd.memset(bxr3[:, :, 0:1], 0.0)
                sl, sr = bxl, bxr

                tmp = tmp_pool.tile([128, TMPSZ], FP8, tag="tmp",
                                    name=f"tmp_{s}")
                nc.gpsimd.memset(tmp[:, 0:GUARD], 0.0)
                for k in range(9):
                    gap_end = SLOT_BASE[k + 1] if k < 8 else TMPSZ
                    nc.gpsimd.memset(
                        tmp[:, SLOT_BASE[k] + SECF:gap_end], 0.0)

                srcs = {0: sl, 1: bxs, 2: sr}
                for (u, v), slot in TAPS:
                    woff = (s * 9 + slot) * W
                    wv = wall[:, woff:woff + W].bitcast(I32).unsqueeze(1) \
                        .broadcast_to([128, PH, W // 4])
                    sv = srcs[v][:].rearrange("p (r w) -> p r w", r=PH) \
                        .bitcast(I32)
                    dst = _slot_ap(tmp, slot, 0, [[W, PH], [1, W]]).bitcast(I32)
                    nc.vector.scalar_tensor_tensor(dst, wv, maskt[:, 0:1], sv,
                                                   AND, XOR)
                tmps_by_s[s] = tmp

            def stage_tapsum(s):
                tmp = tmps_by_s.pop(s)
                bq = bsaq_pool.tile([128, SECF], FP8, tag="bsaq", bufs=3,
                                    name=f"bsaq_{s}")
                bsaq.append(bq)
                for c in range(NCHUNK):
                    o0 = c * CH
                    ps = pstap_pool.tile([128, CH], F32, tag="dwps", bufs=2,
                                         name=f"dwp_{s}_{c}")
                    first = True
                    # only the even-base DR pair B ((0,1),(2,1)); odd-base
                    # pairs intermittently misread, so those taps are singles
                    for a in (2,):
                        (ua, va), _ = TAPS[a]
                        (ub, vb), _ = TAPS[a + 1]
                        da, db = DELTA[(ua, va)], DELTA[(ub, vb)]
                        step = SLOT_BASE[a + 1] - SLOT_BASE[a] + db - da
                        rhs = _slot_ap(tmp, a, da + o0, [[step, 2], [1, CH]])
                        nc.tensor.matmul(ps[:], id2dr, rhs, start=first,
                                         stop=False,
                                         perf_mode=mybir.MatmulPerfMode.DoubleRow)
                        first = False
                    for slot in (0, 1, 4, 5, 6, 7, 8):
                        (u, v), _ = TAPS[slot]
                        rhs1 = _slot_ap(tmp, slot, DELTA[(u, v)] + o0,
                                        [[1, CH]])
                        nc.tensor.matmul(ps[:], id1t[:], rhs1, start=first,
                                         stop=(slot == 8))
                        first = False
                    nc.scalar.activation(bq[:, o0:o0 + CH], ps[:], SIGN)

            def emit_bsa2(g):
                """[128, 18*226] fp8 strip tile: half0 rows -1..16 (slots
                0..17), half1 rows 0..16 (slots 0..16), zero guard cols."""
                t2 = bsa2_pool.tile([128, 18 * ROWP], FP8, tag="bsa2",
                                    name=f"bsa2_{g}")
                t3 = t2[:].rearrange("p (r w) -> p r w", r=18)
                nc.gpsimd.memset(t3[:, :, 0:1], 0.0)
                nc.gpsimd.memset(t3[:, :, ROWP - 1:ROWP], 0.0)
                sg, hb = divmod(g, 2)
                hb *= 64
                bq3 = bsaq[sg][:].rearrange("p (ph w) -> p ph w", ph=PH)
                nc.sync.dma_start(t3[0:64, 1:17, 1:W + 1], bq3[hb:hb + 64, :, :])
                nc.gpsimd.dma_start(t3[64:128, 0:16, 1:W + 1],
                                    bq3[hb:hb + 64, :, :])
                if g > 0:
                    sp, hp = divmod(g - 1, 2)
                    bp3 = bsaq[sp][:].rearrange("p (ph w) -> p ph w", ph=PH)
                    nc.sync.dma_start(t3[0:64, 0:1, 1:W + 1],
                                      bp3[hp * 64:hp * 64 + 64, 15:16, :])
                else:
                    nc.gpsimd.memset(t3[0:64, 0:1, :], 0.0)
                if g < NSTRIP - 1:
                    sn, hn = divmod(g + 1, 2)
                    bn3 = bsaq[sn][:].rearrange("p (ph w) -> p ph w", ph=PH)
                    nc.sync.dma_start(t3[0:64, 17:18, 1:W + 1],
                                      bn3[hn * 64:hn * 64 + 64, 0:1, :])
                    nc.gpsimd.dma_start(t3[64:128, 16:17, 1:W + 1],
                                        bn3[hn * 64:hn * 64 + 64, 0:1, :])
                else:
                    nc.gpsimd.memset(t3[0:64, 17:18, :], 0.0)
                    nc.gpsimd.memset(t3[64:128, 16:17, :], 0.0)
                # half1 slot 17 is read (x0 weight) by the kh2-kw0 pair MM;
                # zero it so stray NaN bytes can't poison PSUM via NaN*0.
                nc.gpsimd.memset(t3[64:128, 17:18, :], 0.0)
                return t2

            def emit_strip(g, t2):
                """Main conv for one strip: 4 fp8 matmuls per 2-row tile."""
                t3 = t2[:].rearrange("p (r w) -> p r w", r=18)
                ob = out_pool.tile([CO, 8 * CH], F16, tag="out", name=f"ob_{g}")
                for rh in range(2):
                    rps = range(4 * rh, 4 * rh + 4)
                    ps = {rp: psconv_pool.tile([CO, 2 * W], F32, tag="cps",
                                               bufs=6, name=f"ps_{g}_{rp}")
                          for rp in rps}
                    ps3 = {rp: ps[rp][:].rearrange("p (r w) -> p r w", r=2)
                           for rp in rps}

                    def rhs4(base_p, npart, off, pair_step):
                        a = t2[base_p:base_p + npart, :]
                        return bass.AP(tensor=a.tensor, offset=a.offset + off,
                                       ap=[list(a.ap[0]), [pair_step, 2],
                                           [ROWP, 2], [1, W]])

                    for rp in rps:   # kh01 x kw01, K=128 DR
                        nc.tensor.matmul(ps3[rp], w2a_dr,
                                         rhs4(0, 128, 2 * rp * ROWP, 1),
                                         start=True, stop=False,
                                         perf_mode=mybir.MatmulPerfMode.DoubleRow)
                    for rp in rps:   # (kh01-kw2, kh2-kw0), K=128 DR
                        nc.tensor.matmul(ps3[rp], w2b2_dr,
                                         rhs4(0, 128, 2 * rp * ROWP + 2,
                                              2 * ROWP - 2),
                                         start=False, stop=False,
                                         perf_mode=mybir.MatmulPerfMode.DoubleRow)
                    for rp in rps:   # kh2-kw1, K=64 single
                        nc.tensor.matmul(ps3[rp], w2c1,
                                         t3[64:128, 2 * rp + 1:2 * rp + 3,
                                            1:1 + W],
                                         start=False, stop=False)
                    for rp in rps:   # kh2-kw2, K=64 single
                        nc.tensor.matmul(ps3[rp], w2c2,
                                         t3[64:128, 2 * rp + 1:2 * rp + 3,
                                            2:2 + W],
                                         start=False, stop=True)
                    for rp in rps:
                        dst = ob[:, rp * CH:(rp + 1) * CH]
                        if rp in (0, 4):
                            nc.scalar.copy(dst, ps[rp][:])
                        else:
                            nc.vector.tensor_copy(dst, ps[rp][:])
                nc.gpsimd.dma_start(y[g, :, :], ob[:])

            stage_front(0)
            stage_front(1)
            for s in range(NSEC):
                stage_tapsum(s)
                t2s = [(g, emit_bsa2(g)) for g in STRIPS_AFTER[s]]
                if s + 2 < NSEC:
                    stage_front(s + 2)
                for g, t2 in t2s:
                    emit_strip(g, t2)
    _split_multiwait_ctrl(nc)
    return nc


def _host_prep(x, patch_filters, output_filters):
    """Pure layout/gather/cast prep of raw values (no arithmetic)."""
    B = x.shape[0]
    # x: [B, 64, 224, 224] -> per-core [7, 128, 3584] bf16
    xr = np.asarray(x, np.float32).reshape(B, C, NSEC, 2, PH, W)
    xt = np.ascontiguousarray(xr.transpose(0, 2, 3, 1, 4, 5)) \
        .reshape(B, NSEC, 128, SECF).astype(ml_dtypes.bfloat16)

    # depthwise weights: [64, 196, 3, 3] raw -> fp8 sign-preserving cast,
    # expanded per column: wdw[p=(h,c), (s,t_slot,col)]
    pf = np.asarray(patch_filters, np.float32).reshape(C, GW, GW, 3, 3)
    wdw = np.zeros((128, NSEC, 9, GW, PH), np.float32)
    for (u, v), slot in TAPS:
        # [c, gy, gx] -> [s, h, c, gx]
        wv = pf[:, :, :, u, v].reshape(C, NSEC, 2, GW).transpose(1, 2, 0, 3)
        wdw[:, :, slot, :, 0] = wv.reshape(NSEC, 128, GW).transpose(1, 0, 2)
    wdw[:] = wdw[:, :, :, :, 0:1]          # expand over the 16 patch cols
    wdw = wdw.reshape(128, NSEC * 9 * W).astype(ml_dtypes.float8_e4m3)

    # main-conv weights (m2 layout)
    of = np.asarray(output_filters, np.float32)
    w2 = np.zeros((128, 6, 128), np.float32)
    for kh in range(2):
        for c in range(C):
            p = 64 * kh + c
            w2[p, 0, :] = of[:, c, kh, 0]      # w2a_dr pair elem 0 (kw0)
            w2[p, 1, :] = of[:, c, kh, 1]      # w2a_dr pair elem 1 (kw1)
            w2[p, 2, :] = of[:, c, kh, 2]      # w2b2_dr elem 0 (kw2)
            w2[p, 3, :] = of[:, c, 2, 0] if kh == 0 else 0.0  # elem 1 (kh2-kw0)
    for c in range(C):
        w2[64 + c, 4, :] = of[:, c, 2, 1]      # kh2-kw1 single
        w2[64 + c, 5, :] = of[:, c, 2, 2]      # kh2-kw2 single
    w2 = w2.reshape(128, 768).astype(ml_dtypes.bfloat16)

    id1 = np.eye(128, dtype=np.float32).astype(ml_dtypes.float8_e4m3)
    id2 = np.zeros((128, 2, 128), np.float32)
    for k in range(128):
        id2[k, :, k] = 1.0
    id2 = id2.reshape(128, 256).astype(ml_dtypes.float8_e4m3)

    in_maps = [{"xt": np.ascontiguousarray(xt[b]), "wdw": wdw, "w2": w2,
                "id2": id2, "id1": id1} for b in range(B)]
    return in_maps


_PROGRAM_CACHE = {}


def _get_program():
    if "nc" not in _PROGRAM_CACHE:
        _PROGRAM_CACHE["nc"] = _build_program()
    return _PROGRAM_CACHE["nc"]


def run(x, a, patch_filters, output_filters, trace=False):
    """Run on 8 NeuronCores (batch-parallel); returns (out, results_obj)."""
    nc = _get_program()
    in_maps = _host_prep(x, patch_filters, output_filters)
    res = run_bass_kernel_spmd(nc, in_maps, list(range(8)), trace=trace)
    outs = []
    for b in range(8):
        yb = np.asarray(res.results[b]["y"], np.float32)  # [14, 128, 8*448]
        yb = yb.reshape(NSTRIP, CO, 8, 2, W).transpose(1, 0, 2, 3, 4)
        outs.append(yb.reshape(CO, H, W))
    return np.stack(outs), res


def kernel(x, a, patch_filters, output_filters):
    out, _ = run(x, a, patch_filters, output_filters)
    return out
